# revision 27
# baseline (speedup 1.0000x reference)
"""Trainium2 Bass kernel for nn_AttentionHead (B=8, T=2048, D=1024, H=64).

Single attention head with additive relative-position scores:
    k = x@Wk + bk; q = x@Wq + bq; v = x@Wv
    S = (q k^T) sqrt(H) + einsum(btc,tvc->btv)(q, rel)  [+ causal mask]
    out = softmax(S) @ v

Distribution: query-block parallel over 8 NeuronCores. Core c owns query
blocks {c, 15-c} (128 rows each) so causal work is balanced. One SPMD
program runs on every core; per-core differences (which rel rows, which
causal mask, which q columns) are carried entirely by the input data.

Numerics: the PE's fp32 matmul path is only ~tf32 accurate, which is not
enough for the large-magnitude logits here (softmax near-ties amplify
score error). The q/k score-path matmuls therefore run as bf16 hi/lo
split products (error ~2^-17); x and W are split on the host. rel runs
in single bf16 (its logit contribution is small, ~5e-3 abs error). v and
the P@V reduction run in single bf16 (linear error only).

K/V projection is batch-sharded: core c projects only batch b=c of x,
then two HBM AllGathers replicate kstack/vnat to every core. This cuts
the projection matmul work and the x stream 8x vs the replicated form.
"""

import os
from contextlib import ExitStack

import numpy as np
import ml_dtypes

import concourse.bass as bass
import concourse.tile as tile
from concourse import bacc, mybir
from concourse.bass_utils import run_bass_kernel_spmd

BF16 = mybir.dt.bfloat16
F16 = mybir.dt.float16
F32 = mybir.dt.float32
F8E4 = mybir.dt.float8e4

# problem shape (hardcoded per contract)
B, T, D, H = 8, 2048, 1024, 64
TB = 128              # query-block rows
NBLK = T // TB        # 16
NCORES = 8
NEG = -240.0

LAST_EXEC_NS = None
LAST_RES = None


def _install_ntff_hook():
    """This image's antenv lacks axon_hooks; synthesize the module and
    register the ctypes NTFF profile hook so trace=True returns timing."""
    import sys
    import types
    try:
        import antenv.axon_hooks  # noqa: F401
        return
    except ImportError:
        pass
    try:
        import antenv
        from trn_agent_boot.trn_boot import _ntff_profile_via_ctypes
        hook = _ntff_profile_via_ctypes("/opt/axon/libaxon_pjrt.so")
        mod = types.ModuleType("antenv.axon_hooks")
        mod._hook = hook
        mod.get_axon_ntff_profile_hook = lambda: mod._hook

        def _set(h):
            mod._hook = h

        mod.set_axon_ntff_profile_hook = _set
        antenv.axon_hooks = mod
        sys.modules["antenv.axon_hooks"] = mod
    except Exception:
        pass


def _cfg(causal: bool):
    # per-core uniform padded extents for the (small, big) block slots
    if causal:
        exts = (1024, 2048)
    else:
        exts = (2048, 2048)
    return {
        "B": B, "T": T, "D": D, "H": H, "TB": TB,
        "exts": exts, "smax": T,
    }


def build_nc(cfg):
    Bc, Tc, Dc, Hc, TBc = cfg["B"], cfg["T"], cfg["D"], cfg["H"], cfg["TB"]
    exts = cfg["exts"]
    smax = cfg["smax"]
    ND = Dc // 128                 # d-tiles
    NQ = 2 * TBc                   # own query rows (2 blocks)
    SCH = 512                      # s-chunk for scores
    SCHP = 256                     # s-chunk for projections
    NPAIR = TBc // 2               # 64 t-pairs per block
    NGRP = NPAIR // 4              # 16 groups of 4 pairs

    nc = bacc.Bacc("TRN2", target_bir_lowering=False, debug=False,
                   num_devices=NCORES)

    NST = smax // 128              # s-tiles for V

    # ---- I/O ----
    # x for k/v projection: only this core's batch slice (b = core id)
    xh = nc.dram_tensor("xh", [Dc, Tc], BF16, kind="ExternalInput")
    xl = nc.dram_tensor("xl", [Dc, Tc], BF16, kind="ExternalInput")
    xqh = nc.dram_tensor("xqh", [Dc, 2, Bc, TBc], BF16, kind="ExternalInput")
    xql = nc.dram_tensor("xql", [Dc, 2, Bc, TBc], BF16, kind="ExternalInput")
    wkh = nc.dram_tensor("wkh", [Dc, Hc], BF16, kind="ExternalInput")
    wkl = nc.dram_tensor("wkl", [Dc, Hc], BF16, kind="ExternalInput")
    wqh = nc.dram_tensor("wqh", [Dc, Hc], BF16, kind="ExternalInput")
    wql = nc.dram_tensor("wql", [Dc, Hc], BF16, kind="ExternalInput")
    wv = nc.dram_tensor("wv", [Dc, Hc], BF16, kind="ExternalInput")
    bk8 = nc.dram_tensor("bk8", [Hc, 1], F32, kind="ExternalInput")
    bq_ = nc.dram_tensor("bq", [Hc, 1], F32, kind="ExternalInput")
    NTILE = (exts[0] + exts[1]) // 512 * (TBc // 8)
    relh = nc.dram_tensor("relh", [NTILE, 128, 2048], BF16,
                          kind="ExternalInput")
    maskA = nc.dram_tensor("maskA", [TBc, exts[0]], F8E4, kind="ExternalInput")
    maskB = nc.dram_tensor("maskB", [TBc, exts[1]], F8E4, kind="ExternalInput")
    identf = nc.dram_tensor("identf", [64, 64], F32, kind="ExternalInput")
    identb = nc.dram_tensor("identb", [128, 128], F16, kind="ExternalInput")
    out = nc.dram_tensor("out", [Bc, 2, TBc, Hc], F32, kind="ExternalOutput")

    # DRAM staging for the k/v all-gather (per-core slice -> all cores)
    kg_in = nc.dram_tensor("kg_in", [128, smax], BF16)
    kg_out = nc.dram_tensor("kg_out", [NCORES, 128, smax], BF16,
                            addr_space="Shared")
    vg_in = nc.dram_tensor("vg_in", [128, NST * Hc], F16)
    vg_out = nc.dram_tensor("vg_out", [NCORES, 128, NST * Hc], F16,
                            addr_space="Shared")
    with tile.TileContext(nc) as tc:
        # ---------------- persistent tiles ----------------
        with (
            tc.tile_pool(name="persist", bufs=1) as pp,
            tc.tile_pool(name="weights", bufs=1) as pw,
            tc.tile_pool(name="Spool", bufs=1) as pS,
            tc.tile_pool(name="Ppool", bufs=1) as pP,
            tc.tile_pool(name="PTpool", bufs=2) as pPT,
            tc.tile_pool(name="outpool", bufs=2) as po,
            tc.tile_pool(name="stats", bufs=4) as pstat,
            tc.tile_pool(name="psmm512", bufs=2, space="PSUM") as ppmm,
            tc.tile_pool(name="psv", bufs=1, space="PSUM") as ppv,
            tc.tile_pool(name="psrel", bufs=2, space="PSUM") as ppr,
            tc.tile_pool(name="pssmall", bufs=2, space="PSUM") as ppsm,
            tc.tile_pool(name="pspt", bufs=1, space="PSUM") as pppt,
        ):
            # q stacks: cols (blk, b, t)
            qmain = pp.tile([128, NQ * Bc], BF16, tag="qmain")  # hi top
            qcorr = pp.tile([128, NQ * Bc], BF16, tag="qcorr")  # lo top
            mA = pp.tile([TBc, exts[0]], F8E4, tag="maskA")
            mB = pp.tile([TBc, exts[1]], F8E4, tag="maskB")
            idf = pw.tile([64, 64], F32, tag="identf")
            idb = pw.tile([128, 128], F16, tag="identb")
            wk_t = pw.tile([128, ND, 2, Hc], BF16, tag="wk")
            wq_t = pw.tile([128, ND, 2, Hc], BF16, tag="wq")
            wv_t = pw.tile([128, ND, Hc], BF16, tag="wv")
            bk_t = pw.tile([Hc, 1], F32, tag="bk")
            bq_t = pw.tile([Hc, 1], F32, tag="bq")

            nc.sync.dma_start(mA, maskA.ap())
            nc.sync.dma_start(mB, maskB.ap())
            nc.sync.dma_start(idf, identf.ap())
            nc.sync.dma_start(idb, identb.ap())
            nc.sync.dma_start(
                wk_t[:, :, 0, :], wkh.ap().rearrange("(n p) h -> p n h", p=128))
            nc.sync.dma_start(
                wk_t[:, :, 1, :], wkl.ap().rearrange("(n p) h -> p n h", p=128))
            nc.sync.dma_start(
                wq_t[:, :, 0, :], wqh.ap().rearrange("(n p) h -> p n h", p=128))
            nc.sync.dma_start(
                wq_t[:, :, 1, :], wql.ap().rearrange("(n p) h -> p n h", p=128))
            nc.sync.dma_start(
                wv_t, wv.ap().rearrange("(n p) h -> p n h", p=128))
            nc.sync.dma_start(bk_t, bk8.ap())
            nc.sync.dma_start(bq_t, bq_.ap())

            # One S tile per block: [t-rows, (b, v)]
            S_all = [pS.tile([TBc, Bc * exts[blk]], F32, tag=f"S{blk}",
                             name=f"S_{blk}")
                     for blk in range(2)]

            # ---------------- streaming phases (interleaved) ----------------
            with ExitStack() as stk:
                ent = stk.enter_context
                pxh = ent(tc.tile_pool(name="xhstream", bufs=2))
                pxl = ent(tc.tile_pool(name="xlstream", bufs=2))
                pst = ent(tc.tile_pool(name="pstage", bufs=2))
                pbd = ent(tc.tile_pool(name="bd", bufs=1))
                prel = ent(tc.tile_pool(name="relstream", bufs=6))
                pstage = ent(tc.tile_pool(name="stage", bufs=2))
                ppush = ent(tc.tile_pool(name="push", bufs=2))

                # ---- q projection over own columns: cols (blk, b, t) ----
                xqf = xqh.ap().rearrange("(n p) k b t -> p n (k b t)", p=128)
                xqlf = xql.ap().rearrange("(n p) k b t -> p n (k b t)", p=128)
                for ci in range(2 * Bc * TBc // SCHP):
                    c0 = ci * SCHP
                    xht = pxh.tile([128, ND, SCHP], BF16, tag="xh")
                    xlt = pxl.tile([128, ND, SCHP], BF16, tag="xl")
                    nc.sync.dma_start(xht, xqf[:, :, c0:c0 + SCHP])
                    nc.sync.dma_start(xlt, xqlf[:, :, c0:c0 + SCHP])
                    psq = ppmm.tile([Hc, SCHP], F32, tag="mm512")
                    for dt_ in range(ND):
                        nc.tensor.matmul(psq, wq_t[:, dt_, 0, :], xht[:, dt_],
                                         start=(dt_ == 0), stop=False)
                    for dt_ in range(ND):
                        nc.tensor.matmul(psq, wq_t[:, dt_, 1, :], xht[:, dt_],
                                         start=False, stop=False)
                    for dt_ in range(ND):
                        nc.tensor.matmul(psq, wq_t[:, dt_, 0, :], xlt[:, dt_],
                                         start=False, stop=(dt_ == ND - 1))
                    qtmp = pst.tile([Hc, SCHP], F32, tag="ktmp")
                    nc.scalar.activation(qtmp, psq,
                                         mybir.ActivationFunctionType.Identity,
                                         bias=bq_t[:, :], scale=1.0)
                    nc.vector.tensor_copy(qmain[0:Hc, c0:c0 + SCHP], qtmp)
                    nc.vector.tensor_tensor(
                        qmain[Hc:128, c0:c0 + SCHP], qtmp,
                        qmain[0:Hc, c0:c0 + SCHP], mybir.AluOpType.subtract)
                    nc.vector.tensor_copy(qcorr[Hc:128, c0:c0 + SCHP],
                                          qmain[0:Hc, c0:c0 + SCHP])
                    nc.vector.tensor_copy(qcorr[0:Hc, c0:c0 + SCHP],
                                          qmain[Hc:128, c0:c0 + SCHP])

                # ---- block-diagonal q tiles for both blocks ----
                bds = []
                for blk in range(2):
                    bdh = pbd.tile([128, NPAIR * 16], BF16, tag="bdh",
                                   name=f"bdh_{blk}")
                    bdl = pbd.tile([128, NPAIR * 16], BF16, tag="bdl",
                                   name=f"bdl_{blk}")
                    nc.vector.memset(bdh, 0.0)
                    nc.vector.memset(bdl, 0.0)
                    qblk_h = (qmain[0:Hc, blk * Bc * TBc:(blk + 1) * Bc * TBc]
                              .rearrange("c (b t) -> c b t", b=Bc))
                    qblk_l = (qmain[Hc:128, blk * Bc * TBc:(blk + 1) * Bc * TBc]
                              .rearrange("c (b t) -> c b t", b=Bc))
                    for j in range(2):
                        dst_h = (bdh[j * Hc:(j + 1) * Hc]
                                 .rearrange("c (p s) -> c p s", s=16)
                                 [:, :, j * 8:j * 8 + 8])
                        src_h = qblk_h[:, :, j::2].rearrange("c b p -> c p b")
                        nc.vector.tensor_copy(dst_h, src_h)
                        dst_l = (bdl[j * Hc:(j + 1) * Hc]
                                 .rearrange("c (p s) -> c p s", s=16)
                                 [:, :, j * 8:j * 8 + 8])
                        src_l = qblk_l[:, :, j::2].rearrange("c b p -> c p b")
                        nc.vector.tensor_copy(dst_l, src_l)
                    bds.append((bdh, bdl))

                # ---- generator: k/v projection (own batch), pushed to DRAM
                def proj_steps():
                    xf = xh.ap().rearrange("(n p) t -> p n t", p=128)
                    xlf = xl.ap().rearrange("(n p) t -> p n t", p=128)
                    for ci in range(smax // SCHP):
                        c0 = ci * SCHP
                        xht = pxh.tile([128, ND, SCHP], BF16, tag="xh")
                        xlt = pxl.tile([128, ND, SCHP], BF16, tag="xl")
                        nc.sync.dma_start(xht, xf[:, :, c0:c0 + SCHP])
                        nc.sync.dma_start(xlt, xlf[:, :, c0:c0 + SCHP])
                        psk = ppmm.tile([Hc, SCHP], F32, tag="mm512")
                        for dt_ in range(ND):
                            nc.tensor.matmul(psk, wk_t[:, dt_, 0, :],
                                             xht[:, dt_],
                                             start=(dt_ == 0), stop=False)
                        for dt_ in range(ND):
                            nc.tensor.matmul(psk, wk_t[:, dt_, 1, :],
                                             xht[:, dt_],
                                             start=False, stop=False)
                        for dt_ in range(ND):
                            nc.tensor.matmul(psk, wk_t[:, dt_, 0, :],
                                             xlt[:, dt_],
                                             start=False, stop=(dt_ == ND - 1))
                        ktmp = pst.tile([Hc, SCHP], F32, tag="ktmp")
                        nc.scalar.activation(
                            ktmp, psk, mybir.ActivationFunctionType.Identity,
                            bias=bk_t[:, :], scale=1.0)
                        kpush = ppush.tile([128, SCHP], BF16, tag="kpush")
                        nc.vector.tensor_copy(kpush[0:Hc, :], ktmp)
                        nc.vector.tensor_tensor(
                            kpush[Hc:128, :], ktmp, kpush[0:Hc, :],
                            mybir.AluOpType.subtract)
                        nc.scalar.dma_start(kg_in.ap()[:, c0:c0 + SCHP], kpush)
                        psv = ppv.tile([Hc, SCHP], F32, tag="pv")
                        for dt_ in range(ND):
                            nc.tensor.matmul(psv, wv_t[:, dt_], xht[:, dt_],
                                             start=(dt_ == 0),
                                             stop=(dt_ == ND - 1))
                        vtmp = pst.tile([Hc, SCHP], F32, tag="vtmp")
                        nc.any.tensor_copy(vtmp, psv)
                        vpush = ppush.tile([128, 2 * Hc], F16, tag="vpush")
                        for sub in range(SCHP // 128):
                            pvt = ppsm.tile([128, Hc], F32, tag="small64")
                            nc.tensor.transpose(
                                pvt, vtmp[:, sub * 128:(sub + 1) * 128],
                                idf[0:Hc, 0:Hc])
                            nc.any.tensor_copy(
                                vpush[:, sub * Hc:(sub + 1) * Hc], pvt)
                        st0 = c0 // 128
                        nc.scalar.dma_start(
                            vg_in.ap()[:, st0 * Hc:(st0 + 2) * Hc], vpush)
                        yield
                    yield
                    nc.gpsimd.collective_compute(
                        "AllGather", mybir.AluOpType.bypass,
                        replica_groups=[list(range(NCORES))],
                        ins=[kg_in[:].opt()], outs=[kg_out[:].opt()])
                    nc.gpsimd.collective_compute(
                        "AllGather", mybir.AluOpType.bypass,
                        replica_groups=[list(range(NCORES))],
                        ins=[vg_in[:].opt()], outs=[vg_out[:].opt()])
                    yield

                # ---- generator: rel-score streaming (scatter deferred) ----
                def rel_steps():
                    relf = relh.ap()
                    ridx = 0
                    pending = []

                    def flush():
                        blk_, g_, stg_ = pending.pop()
                        for u in range(4):
                            nc.scalar.dma_start(
                                S_all[blk_][8 * g_ + 2 * u:
                                            8 * g_ + 2 * u + 2, :],
                                stg_[32 * u:32 * u + 16, :])

                    for blk in range(2):
                        ext = exts[blk]
                        nch = ext // SCH
                        bdh, bdl = bds[blk]
                        for g in range(NGRP):
                            stg = pstage.tile([128, ext], F32, tag="stage")
                            for ch in range(nch):
                                v0 = ch * SCH
                                psr = ppr.tile([128, SCH], F32, tag="pr")
                                rht = prel.tile([128, 4, SCH], BF16, tag="rh")
                                nc.sync.dma_start(rht, relf[ridx])
                                ridx += 1
                                for u in range(4):
                                    p = 4 * g + u
                                    pslice = psr[32 * u:32 * u + 16, :]
                                    bd_h = bdh[:, p * 16:p * 16 + 16]
                                    bd_l = bdl[:, p * 16:p * 16 + 16]
                                    nc.tensor.matmul(pslice, bd_h, rht[:, u],
                                                     start=True, stop=False,
                                                     tile_position=(0, 32 * u))
                                    nc.tensor.matmul(pslice, bd_l, rht[:, u],
                                                     start=False, stop=True,
                                                     tile_position=(0, 32 * u))
                                nc.vector.tensor_copy(stg[:, v0:v0 + SCH], psr)
                                yield
                            if pending:
                                flush()
                            pending.append((blk, g, stg))
                    flush()

                # ---- drive the two streams interleaved (3 rel : 1 proj) ----
                pgen = proj_steps()
                rgen = rel_steps()
                done_p = done_r = False
                while not (done_p and done_r):
                    if not done_p:
                        done_p = next(pgen, "end") == "end"
                    for _ in range(3):
                        if not done_r:
                            done_r = next(rgen, "end") == "end"

            # ---- gather k/v into SBUF (frees the stream pools first) ----
            with tc.tile_pool(name="kv", bufs=1) as pkv:
                kstack = pkv.tile([128, Bc * smax], BF16, tag="kstack")
                vnat = pkv.tile([128, Bc * NST * Hc], F16, tag="vnat")
                for b in range(Bc):
                    nc.gpsimd.dma_start(
                        kstack[:, b * smax:(b + 1) * smax], kg_out.ap()[b])
                    nc.gpsimd.dma_start(
                        vnat[:, b * NST * Hc:(b + 1) * NST * Hc],
                        vg_out.ap()[b])

                # ---- per (block, batch): qk scores, softmax, P^T, AV ----
                for blk in range(2):
                    ext = exts[blk]
                    nch = ext // SCH
                    msk = mA if blk == 0 else mB
                    for b in range(Bc):
                        S = S_all[blk][:, b * ext:(b + 1) * ext]
                        qm = qmain[0:Hc, (blk * Bc + b) * TBc:
                                   (blk * Bc + b + 1) * TBc]
                        qc = qcorr[:, (blk * Bc + b) * TBc:
                                   (blk * Bc + b + 1) * TBc]
                        # mask pass, then QK accumulate per chunk
                        nc.vector.tensor_tensor(S, S, msk,
                                                mybir.AluOpType.add)
                        for ch in range(nch):
                            s0 = ch * SCH
                            psS = ppmm.tile([TBc, SCH], F32, tag="mm512")
                            cols = slice(b * smax + s0, b * smax + s0 + SCH)
                            nc.tensor.matmul(psS, qm, kstack[0:Hc, cols],
                                             start=True, stop=False)
                            nc.tensor.matmul(psS, qc, kstack[:, cols],
                                             start=False, stop=True)
                            nc.vector.tensor_tensor(
                                S[:, s0:s0 + SCH], psS, S[:, s0:s0 + SCH],
                                mybir.AluOpType.add)
                        negmax = pstat.tile([TBc, 1], F32, tag="negmax")
                        zsum = pstat.tile([TBc, 1], F32, tag="zsum")
                        rz = pstat.tile([TBc, 1], F32, tag="rz")
                        nc.vector.tensor_reduce(negmax, S,
                                                mybir.AxisListType.X,
                                                mybir.AluOpType.max,
                                                negate=True)
                        P = pP.tile([TBc, ext], F16, tag="P")
                        nc.scalar.activation(P, S,
                                             mybir.ActivationFunctionType.Exp,
                                             bias=negmax[:, :], scale=1.0,
                                             accum_out=zsum[:, :])
                        nc.vector.reciprocal(rz, zsum)
                        pso = ppsm.tile([TBc, Hc], F32, tag="small64")
                        for st in range(ext // 128):
                            ppt = pppt.tile([128, 128], F16, tag="pt")
                            nc.tensor.transpose(
                                ppt, P[:, st * 128:(st + 1) * 128], idb)
                            ptt = pPT.tile([128, 128], F16, tag="ptt")
                            nc.any.tensor_copy(ptt, ppt)
                            nc.tensor.matmul(
                                pso, ptt,
                                vnat[:, (b * NST + st) * Hc:
                                     (b * NST + st + 1) * Hc],
                                start=(st == 0), stop=(st == ext // 128 - 1))
                        osb = po.tile([TBc, Hc], F32, tag="osb")
                        nc.vector.tensor_scalar_mul(osb, pso, rz[:, :])
                        nc.sync.dma_start(out.ap()[b, blk], osb)

    nc.compile()
    return nc


def _split(a):
    hi = np.asarray(a, dtype=np.float32).astype(ml_dtypes.bfloat16)
    lo = (np.asarray(a, dtype=np.float32) - hi.astype(np.float32)).astype(
        ml_dtypes.bfloat16)
    return hi, lo


def kernel(x, Wk, bk, Wq, bq, Wv, rel_pos_emb, mask, **_unused):
    global LAST_EXEC_NS
    x = np.asarray(x, dtype=np.float32)
    Wk = np.asarray(Wk, dtype=np.float32)
    bk = np.asarray(bk, dtype=np.float32)
    Wq = np.asarray(Wq, dtype=np.float32)
    bq = np.asarray(bq, dtype=np.float32)
    Wv = np.asarray(Wv, dtype=np.float32)
    rel = np.asarray(rel_pos_emb, dtype=np.float32)
    causal = bool(np.asarray(mask).item())
    cfg = _cfg(causal)
    exts = cfg["exts"]

    scale = np.float32(np.sqrt(H))
    # xT: [D, B, T]
    xT = np.ascontiguousarray(x.transpose(2, 0, 1))
    xh, xl = _split(xT)
    wkh, wkl = _split(Wk * scale)
    wqh, wql = _split(Wq)
    wvh = Wv.astype(ml_dtypes.bfloat16)
    bk8 = (bk * scale).reshape(H, 1).astype(np.float32)
    bqr = bq.reshape(H, 1).astype(np.float32)
    # relT: [T, H, T] (t, c, v) — single bf16 (no hi/lo split)
    relT = np.ascontiguousarray(rel.transpose(0, 2, 1))
    rth = relT.astype(ml_dtypes.bfloat16)
    identf = np.eye(64, dtype=np.float32)
    identb = np.eye(128, dtype=np.float16)

    in_maps = []
    blocks = []
    for c in range(NCORES):
        bA, bB = c, NBLK - 1 - c
        blocks.append((bA, bB))
        tiles = []
        for slot, blkid in ((0, bA), (1, bB)):
            rblk = rth[blkid * TB:(blkid + 1) * TB]       # [128, H, T]
            for g in range(TB // 8):
                rows = rblk[8 * g:8 * g + 8]              # [8, H, T]
                for ch in range(exts[slot] // 512):
                    t = rows[:, :, ch * 512:(ch + 1) * 512]
                    t = (t.reshape(4, 2, H, 512)
                         .transpose(1, 2, 0, 3).reshape(128, 2048))
                    tiles.append(t)
        relh_c = np.stack(tiles)
        xqh_c = np.stack([xh[:, :, bA * TB:(bA + 1) * TB],
                          xh[:, :, bB * TB:(bB + 1) * TB]], axis=1)
        xql_c = np.stack([xl[:, :, bA * TB:(bA + 1) * TB],
                          xl[:, :, bB * TB:(bB + 1) * TB]], axis=1)
        masks = []
        for slot, blkid in ((0, bA), (1, bB)):
            ext = exts[slot]
            t_idx = blkid * TB + np.arange(TB)[:, None]
            s_idx = np.arange(ext)[None, :]
            if causal:
                m = np.where(s_idx <= t_idx, 0.0, NEG)
            else:
                m = np.zeros((TB, ext))
            masks.append(np.ascontiguousarray(
                m.astype(ml_dtypes.float8_e4m3)))
        in_maps.append({
            "xh": np.ascontiguousarray(xh[:, c, :]),
            "xl": np.ascontiguousarray(xl[:, c, :]),
            "xqh": np.ascontiguousarray(xqh_c),
            "xql": np.ascontiguousarray(xql_c),
            "wkh": wkh, "wkl": wkl, "wqh": wqh, "wql": wql, "wv": wvh,
            "bk8": bk8, "bq": bqr,
            "relh": np.ascontiguousarray(relh_c),
            "maskA": masks[0], "maskB": masks[1],
            "identf": identf, "identb": identb,
        })

    nc = build_nc(cfg)
    if os.environ.get("KERNEL_TRACE") == "1":
        # the profile hook needs an initialized backend; trigger init first.
        _install_ntff_hook()
        import jax
        jax.devices()
        try:
            res = run_bass_kernel_spmd(
                nc, in_maps, core_ids=list(range(NCORES)), trace=True)
        except Exception:
            res = run_bass_kernel_spmd(
                nc, in_maps, core_ids=list(range(NCORES)))
    else:
        res = run_bass_kernel_spmd(nc, in_maps, core_ids=list(range(NCORES)))
    LAST_EXEC_NS = res.exec_time_ns
    global LAST_RES
    LAST_RES = res

    out = np.empty((B, T, H), dtype=np.float32)
    for c in range(NCORES):
        oc = res.results[c]["out"]          # [B, 2, TB, H]
        bA, bB = blocks[c]
        out[:, bA * TB:(bA + 1) * TB] = oc[:, 0]
        out[:, bB * TB:(bB + 1) * TB] = oc[:, 1]
    return out



# revision 29
# speedup vs baseline: 1.0405x; 1.0405x over previous
"""Trainium2 Bass kernel for nn_AttentionHead (B=8, T=2048, D=1024, H=64).

Single attention head with additive relative-position scores:
    k = x@Wk + bk; q = x@Wq + bq; v = x@Wv
    S = (q k^T) sqrt(H) + einsum(btc,tvc->btv)(q, rel)  [+ causal mask]
    out = softmax(S) @ v

Distribution: query-block parallel over 8 NeuronCores. Core c owns query
blocks {c, 15-c} (128 rows each) so causal work is balanced. One SPMD
program runs on every core; per-core differences (which rel rows, which
causal mask, which q columns) are carried entirely by the input data.

Numerics: the PE's fp32 matmul path is only ~tf32 accurate, which is not
enough for the large-magnitude logits here (softmax near-ties amplify
score error). The q/k score-path matmuls therefore run as bf16 hi/lo
split products (error ~2^-17); x and W are split on the host. rel runs
in single bf16 (its logit contribution is small, ~5e-3 abs error). v and
the P@V reduction run in single bf16 (linear error only).

K/V projection is batch-sharded: core c projects only batch b=c of x,
then two HBM AllGathers replicate kstack/vnat to every core. This cuts
the projection matmul work and the x stream 8x vs the replicated form.
"""

import os
from contextlib import ExitStack

import numpy as np
import ml_dtypes

import concourse.bass as bass
import concourse.tile as tile
from concourse import bacc, mybir
from concourse.bass_utils import run_bass_kernel_spmd

BF16 = mybir.dt.bfloat16
F16 = mybir.dt.float16
F32 = mybir.dt.float32
F8E4 = mybir.dt.float8e4

# problem shape (hardcoded per contract)
B, T, D, H = 8, 2048, 1024, 64
TB = 128              # query-block rows
NBLK = T // TB        # 16
NCORES = 8
NEG = -240.0

LAST_EXEC_NS = None
LAST_RES = None


def _install_ntff_hook():
    """This image's antenv lacks axon_hooks; synthesize the module and
    register the ctypes NTFF profile hook so trace=True returns timing."""
    import sys
    import types
    try:
        import antenv.axon_hooks  # noqa: F401
        return
    except ImportError:
        pass
    try:
        import antenv
        from trn_agent_boot.trn_boot import _ntff_profile_via_ctypes
        hook = _ntff_profile_via_ctypes("/opt/axon/libaxon_pjrt.so")
        mod = types.ModuleType("antenv.axon_hooks")
        mod._hook = hook
        mod.get_axon_ntff_profile_hook = lambda: mod._hook

        def _set(h):
            mod._hook = h

        mod.set_axon_ntff_profile_hook = _set
        antenv.axon_hooks = mod
        sys.modules["antenv.axon_hooks"] = mod
    except Exception:
        pass


def _cfg(causal: bool):
    # per-core uniform padded extents for the (small, big) block slots
    if causal:
        exts = (1024, 2048)
    else:
        exts = (2048, 2048)
    return {
        "B": B, "T": T, "D": D, "H": H, "TB": TB,
        "exts": exts, "smax": T,
    }


def build_nc(cfg):
    Bc, Tc, Dc, Hc, TBc = cfg["B"], cfg["T"], cfg["D"], cfg["H"], cfg["TB"]
    exts = cfg["exts"]
    smax = cfg["smax"]
    ND = Dc // 128                 # d-tiles
    NQ = 2 * TBc                   # own query rows (2 blocks)
    SCH = 512                      # s-chunk for scores
    SCHP = 256                     # s-chunk for projections
    NPAIR = TBc // 2               # 64 t-pairs per block
    NGRP = NPAIR // 4              # 16 groups of 4 pairs

    nc = bacc.Bacc("TRN2", target_bir_lowering=False, debug=False,
                   num_devices=NCORES)

    NST = smax // 128              # s-tiles for V

    # ---- I/O ----
    # x for k/v projection: only this core's batch slice (b = core id)
    xh = nc.dram_tensor("xh", [Dc, Tc], BF16, kind="ExternalInput")
    xl = nc.dram_tensor("xl", [Dc, Tc], BF16, kind="ExternalInput")
    xqh = nc.dram_tensor("xqh", [Dc, 2, Bc, TBc], BF16, kind="ExternalInput")
    xql = nc.dram_tensor("xql", [Dc, 2, Bc, TBc], BF16, kind="ExternalInput")
    wkh = nc.dram_tensor("wkh", [Dc, Hc], BF16, kind="ExternalInput")
    wkl = nc.dram_tensor("wkl", [Dc, Hc], BF16, kind="ExternalInput")
    wqh = nc.dram_tensor("wqh", [Dc, Hc], BF16, kind="ExternalInput")
    wql = nc.dram_tensor("wql", [Dc, Hc], BF16, kind="ExternalInput")
    wv = nc.dram_tensor("wv", [Dc, Hc], BF16, kind="ExternalInput")
    bk8 = nc.dram_tensor("bk8", [Hc, 1], F32, kind="ExternalInput")
    bq_ = nc.dram_tensor("bq", [Hc, 1], F32, kind="ExternalInput")
    NTILE = (exts[0] + exts[1]) // 512 * (TBc // 8)
    relh = nc.dram_tensor("relh", [NTILE, 128, 2048], BF16,
                          kind="ExternalInput")
    maskA = nc.dram_tensor("maskA", [TBc, exts[0]], F8E4, kind="ExternalInput")
    maskB = nc.dram_tensor("maskB", [TBc, exts[1]], F8E4, kind="ExternalInput")
    identf = nc.dram_tensor("identf", [64, 64], F32, kind="ExternalInput")
    identb = nc.dram_tensor("identb", [128, 128], F16, kind="ExternalInput")
    out = nc.dram_tensor("out", [Bc, 2, TBc, Hc], F32, kind="ExternalOutput")

    # DRAM staging for the k/v all-gather (per-core slice -> all cores)
    kg_in = nc.dram_tensor("kg_in", [128, smax], BF16)
    kg_out = nc.dram_tensor("kg_out", [NCORES, 128, smax], BF16,
                            addr_space="Shared")
    vg_in = nc.dram_tensor("vg_in", [128, NST * Hc], F16)
    vg_out = nc.dram_tensor("vg_out", [NCORES, 128, NST * Hc], F16,
                            addr_space="Shared")
    with tile.TileContext(nc) as tc:
        # ---------------- persistent tiles ----------------
        with (
            tc.tile_pool(name="persist", bufs=1) as pp,
            tc.tile_pool(name="weights", bufs=1) as pw,
            tc.tile_pool(name="Spool", bufs=1) as pS,
            tc.tile_pool(name="Ppool", bufs=1) as pP,
            tc.tile_pool(name="PTpool", bufs=2) as pPT,
            tc.tile_pool(name="outpool", bufs=2) as po,
            tc.tile_pool(name="stats", bufs=4) as pstat,
            tc.tile_pool(name="psmm512", bufs=2, space="PSUM") as ppmm,
            tc.tile_pool(name="psv", bufs=1, space="PSUM") as ppv,
            tc.tile_pool(name="psrel", bufs=2, space="PSUM") as ppr,
            tc.tile_pool(name="pssmall", bufs=2, space="PSUM") as ppsm,
            tc.tile_pool(name="pspt", bufs=1, space="PSUM") as pppt,
        ):
            # q stacks: cols (blk, b, t)
            qmain = pp.tile([128, NQ * Bc], BF16, tag="qmain")  # hi top
            qcorr = pp.tile([128, NQ * Bc], BF16, tag="qcorr")  # lo top
            mA = pp.tile([TBc, exts[0]], F8E4, tag="maskA")
            mB = pp.tile([TBc, exts[1]], F8E4, tag="maskB")
            idf = pw.tile([64, 64], F32, tag="identf")
            idb = pw.tile([128, 128], F16, tag="identb")
            wk_t = pw.tile([128, ND, 2, Hc], BF16, tag="wk")
            wq_t = pw.tile([128, ND, 2, Hc], BF16, tag="wq")
            wv_t = pw.tile([128, ND, Hc], BF16, tag="wv")
            bk_t = pw.tile([Hc, 1], F32, tag="bk")
            bq_t = pw.tile([Hc, 1], F32, tag="bq")

            nc.sync.dma_start(mA, maskA.ap())
            nc.sync.dma_start(mB, maskB.ap())
            nc.sync.dma_start(idf, identf.ap())
            nc.sync.dma_start(idb, identb.ap())
            nc.sync.dma_start(
                wk_t[:, :, 0, :], wkh.ap().rearrange("(n p) h -> p n h", p=128))
            nc.sync.dma_start(
                wk_t[:, :, 1, :], wkl.ap().rearrange("(n p) h -> p n h", p=128))
            nc.sync.dma_start(
                wq_t[:, :, 0, :], wqh.ap().rearrange("(n p) h -> p n h", p=128))
            nc.sync.dma_start(
                wq_t[:, :, 1, :], wql.ap().rearrange("(n p) h -> p n h", p=128))
            nc.sync.dma_start(
                wv_t, wv.ap().rearrange("(n p) h -> p n h", p=128))
            nc.sync.dma_start(bk_t, bk8.ap())
            nc.sync.dma_start(bq_t, bq_.ap())

            # One S tile per block: [t-rows, (b, v)]
            S_all = [pS.tile([TBc, Bc * exts[blk]], F32, tag=f"S{blk}",
                             name=f"S_{blk}")
                     for blk in range(2)]

            # ---------------- streaming phases (interleaved) ----------------
            with ExitStack() as stkA, ExitStack() as stk:
                entA = stkA.enter_context
                pbd = entA(tc.tile_pool(name="bd", bufs=1))
                prel = entA(tc.tile_pool(name="relstream", bufs=3))
                pstage = entA(tc.tile_pool(name="stage", bufs=2))
                ent = stk.enter_context
                pxh = ent(tc.tile_pool(name="xhstream", bufs=2))
                pxl = ent(tc.tile_pool(name="xlstream", bufs=2))
                pst = ent(tc.tile_pool(name="pstage", bufs=2))
                ppush = ent(tc.tile_pool(name="push", bufs=2))

                # ---- q projection over own columns: cols (blk, b, t) ----
                xqf = xqh.ap().rearrange("(n p) k b t -> p n (k b t)", p=128)
                xqlf = xql.ap().rearrange("(n p) k b t -> p n (k b t)", p=128)
                for ci in range(2 * Bc * TBc // SCHP):
                    c0 = ci * SCHP
                    xht = pxh.tile([128, ND, SCHP], BF16, tag="xh")
                    xlt = pxl.tile([128, ND, SCHP], BF16, tag="xl")
                    nc.sync.dma_start(xht, xqf[:, :, c0:c0 + SCHP])
                    nc.sync.dma_start(xlt, xqlf[:, :, c0:c0 + SCHP])
                    psq = ppmm.tile([Hc, SCHP], F32, tag="mm512")
                    for dt_ in range(ND):
                        nc.tensor.matmul(psq, wq_t[:, dt_, 0, :], xht[:, dt_],
                                         start=(dt_ == 0), stop=False)
                    for dt_ in range(ND):
                        nc.tensor.matmul(psq, wq_t[:, dt_, 1, :], xht[:, dt_],
                                         start=False, stop=False)
                    for dt_ in range(ND):
                        nc.tensor.matmul(psq, wq_t[:, dt_, 0, :], xlt[:, dt_],
                                         start=False, stop=(dt_ == ND - 1))
                    qtmp = pst.tile([Hc, SCHP], F32, tag="ktmp")
                    nc.scalar.activation(qtmp, psq,
                                         mybir.ActivationFunctionType.Identity,
                                         bias=bq_t[:, :], scale=1.0)
                    nc.vector.tensor_copy(qmain[0:Hc, c0:c0 + SCHP], qtmp)
                    nc.vector.tensor_tensor(
                        qmain[Hc:128, c0:c0 + SCHP], qtmp,
                        qmain[0:Hc, c0:c0 + SCHP], mybir.AluOpType.subtract)
                    nc.vector.tensor_copy(qcorr[Hc:128, c0:c0 + SCHP],
                                          qmain[0:Hc, c0:c0 + SCHP])
                    nc.vector.tensor_copy(qcorr[0:Hc, c0:c0 + SCHP],
                                          qmain[Hc:128, c0:c0 + SCHP])

                # ---- block-diagonal q tiles for both blocks ----
                bds = []
                for blk in range(2):
                    bdh = pbd.tile([128, NPAIR * 16], BF16, tag="bdh",
                                   name=f"bdh_{blk}")
                    bdl = pbd.tile([128, NPAIR * 16], BF16, tag="bdl",
                                   name=f"bdl_{blk}")
                    nc.vector.memset(bdh, 0.0)
                    nc.vector.memset(bdl, 0.0)
                    qblk_h = (qmain[0:Hc, blk * Bc * TBc:(blk + 1) * Bc * TBc]
                              .rearrange("c (b t) -> c b t", b=Bc))
                    qblk_l = (qmain[Hc:128, blk * Bc * TBc:(blk + 1) * Bc * TBc]
                              .rearrange("c (b t) -> c b t", b=Bc))
                    for j in range(2):
                        dst_h = (bdh[j * Hc:(j + 1) * Hc]
                                 .rearrange("c (p s) -> c p s", s=16)
                                 [:, :, j * 8:j * 8 + 8])
                        src_h = qblk_h[:, :, j::2].rearrange("c b p -> c p b")
                        nc.vector.tensor_copy(dst_h, src_h)
                        dst_l = (bdl[j * Hc:(j + 1) * Hc]
                                 .rearrange("c (p s) -> c p s", s=16)
                                 [:, :, j * 8:j * 8 + 8])
                        src_l = qblk_l[:, :, j::2].rearrange("c b p -> c p b")
                        nc.vector.tensor_copy(dst_l, src_l)
                    bds.append((bdh, bdl))

                # ---- generator: k/v projection (own batch), pushed to DRAM
                def proj_steps():
                    xf = xh.ap().rearrange("(n p) t -> p n t", p=128)
                    xlf = xl.ap().rearrange("(n p) t -> p n t", p=128)
                    for ci in range(smax // SCHP):
                        c0 = ci * SCHP
                        xht = pxh.tile([128, ND, SCHP], BF16, tag="xh")
                        xlt = pxl.tile([128, ND, SCHP], BF16, tag="xl")
                        nc.sync.dma_start(xht, xf[:, :, c0:c0 + SCHP])
                        nc.sync.dma_start(xlt, xlf[:, :, c0:c0 + SCHP])
                        psk = ppmm.tile([Hc, SCHP], F32, tag="mm512")
                        for dt_ in range(ND):
                            nc.tensor.matmul(psk, wk_t[:, dt_, 0, :],
                                             xht[:, dt_],
                                             start=(dt_ == 0), stop=False)
                        for dt_ in range(ND):
                            nc.tensor.matmul(psk, wk_t[:, dt_, 1, :],
                                             xht[:, dt_],
                                             start=False, stop=False)
                        for dt_ in range(ND):
                            nc.tensor.matmul(psk, wk_t[:, dt_, 0, :],
                                             xlt[:, dt_],
                                             start=False, stop=(dt_ == ND - 1))
                        ktmp = pst.tile([Hc, SCHP], F32, tag="ktmp")
                        nc.scalar.activation(
                            ktmp, psk, mybir.ActivationFunctionType.Identity,
                            bias=bk_t[:, :], scale=1.0)
                        kpush = ppush.tile([128, SCHP], BF16, tag="kpush")
                        nc.vector.tensor_copy(kpush[0:Hc, :], ktmp)
                        nc.vector.tensor_tensor(
                            kpush[Hc:128, :], ktmp, kpush[0:Hc, :],
                            mybir.AluOpType.subtract)
                        nc.scalar.dma_start(kg_in.ap()[:, c0:c0 + SCHP], kpush)
                        psv = ppv.tile([Hc, SCHP], F32, tag="pv")
                        for dt_ in range(ND):
                            nc.tensor.matmul(psv, wv_t[:, dt_], xht[:, dt_],
                                             start=(dt_ == 0),
                                             stop=(dt_ == ND - 1))
                        vtmp = pst.tile([Hc, SCHP], F32, tag="vtmp")
                        nc.any.tensor_copy(vtmp, psv)
                        vpush = ppush.tile([128, 2 * Hc], F16, tag="vpush")
                        for sub in range(SCHP // 128):
                            pvt = ppsm.tile([128, Hc], F32, tag="small64")
                            nc.tensor.transpose(
                                pvt, vtmp[:, sub * 128:(sub + 1) * 128],
                                idf[0:Hc, 0:Hc])
                            nc.any.tensor_copy(
                                vpush[:, sub * Hc:(sub + 1) * Hc], pvt)
                        st0 = c0 // 128
                        nc.scalar.dma_start(
                            vg_in.ap()[:, st0 * Hc:(st0 + 2) * Hc], vpush)
                        yield
                    yield
                    nc.gpsimd.collective_compute(
                        "AllGather", mybir.AluOpType.bypass,
                        replica_groups=[list(range(NCORES))],
                        ins=[kg_in[:].opt()], outs=[kg_out[:].opt()])
                    nc.gpsimd.collective_compute(
                        "AllGather", mybir.AluOpType.bypass,
                        replica_groups=[list(range(NCORES))],
                        ins=[vg_in[:].opt()], outs=[vg_out[:].opt()])
                    yield

                # ---- generator: rel-score streaming (scatter deferred) ----
                def rel_steps():
                    relf = relh.ap()
                    ridx = 0
                    pending = []

                    def flush():
                        blk_, g_, stg_ = pending.pop()
                        for u in range(4):
                            nc.scalar.dma_start(
                                S_all[blk_][8 * g_ + 2 * u:
                                            8 * g_ + 2 * u + 2, :],
                                stg_[32 * u:32 * u + 16, :])

                    for blk in range(2):
                        ext = exts[blk]
                        nch = ext // SCH
                        bdh, bdl = bds[blk]
                        for g in range(NGRP):
                            stg = pstage.tile([128, ext], F32, tag="stage")
                            for ch in range(nch):
                                v0 = ch * SCH
                                psr = ppr.tile([128, SCH], F32, tag="pr")
                                rht = prel.tile([128, 4, SCH], BF16, tag="rh")
                                nc.sync.dma_start(rht, relf[ridx])
                                ridx += 1
                                for u in range(4):
                                    p = 4 * g + u
                                    pslice = psr[32 * u:32 * u + 16, :]
                                    bd_h = bdh[:, p * 16:p * 16 + 16]
                                    bd_l = bdl[:, p * 16:p * 16 + 16]
                                    nc.tensor.matmul(pslice, bd_h, rht[:, u],
                                                     start=True, stop=False,
                                                     tile_position=(0, 32 * u))
                                    nc.tensor.matmul(pslice, bd_l, rht[:, u],
                                                     start=False, stop=True,
                                                     tile_position=(0, 32 * u))
                                nc.vector.tensor_copy(stg[:, v0:v0 + SCH], psr)
                                yield
                            if pending:
                                flush()
                            pending.append((blk, g, stg))
                    flush()

                # ---- drive the two streams interleaved (3 rel : 1 proj) ----
                pgen = proj_steps()
                rgen = rel_steps()
                done_p = done_r = False
                while not (done_p and done_r):
                    if not done_p:
                        done_p = next(pgen, "end") == "end"
                    for _ in range(3):
                        if not done_r:
                            done_r = next(rgen, "end") == "end"

                # free the projection stream pools; their SBUF becomes the
                # k/v landing zone so the gather can overlap the rel stream
                stk.close()

                # ---- gather k/v into SBUF ----
                pkv = entA(tc.tile_pool(name="kv", bufs=1))
                kstack = pkv.tile([128, Bc * smax], BF16, tag="kstack")
                vnat = pkv.tile([128, Bc * NST * Hc], F16, tag="vnat")
                for b in range(Bc):
                    nc.gpsimd.dma_start(
                        kstack[:, b * smax:(b + 1) * smax], kg_out.ap()[b])
                    nc.gpsimd.dma_start(
                        vnat[:, b * NST * Hc:(b + 1) * NST * Hc],
                        vg_out.ap()[b])

                # ---- per (block, batch): qk scores, softmax, P^T, AV ----
                for blk in range(2):
                    ext = exts[blk]
                    nch = ext // SCH
                    msk = mA if blk == 0 else mB
                    for b in range(Bc):
                        S = S_all[blk][:, b * ext:(b + 1) * ext]
                        qm = qmain[0:Hc, (blk * Bc + b) * TBc:
                                   (blk * Bc + b + 1) * TBc]
                        qc = qcorr[:, (blk * Bc + b) * TBc:
                                   (blk * Bc + b + 1) * TBc]
                        # mask pass (gpsimd; DVE is the tail bottleneck)
                        nc.gpsimd.tensor_tensor(S, S, msk,
                                                mybir.AluOpType.add)
                        for ch in range(nch):
                            s0 = ch * SCH
                            psS = ppmm.tile([TBc, SCH], F32, tag="mm512")
                            cols = slice(b * smax + s0, b * smax + s0 + SCH)
                            nc.tensor.matmul(psS, qm, kstack[0:Hc, cols],
                                             start=True, stop=False)
                            nc.tensor.matmul(psS, qc, kstack[:, cols],
                                             start=False, stop=True)
                            nc.vector.tensor_tensor(
                                S[:, s0:s0 + SCH], psS, S[:, s0:s0 + SCH],
                                mybir.AluOpType.add)
                        negmax = pstat.tile([TBc, 1], F32, tag="negmax")
                        zsum = pstat.tile([TBc, 1], F32, tag="zsum")
                        rz = pstat.tile([TBc, 1], F32, tag="rz")
                        nc.vector.tensor_reduce(negmax, S,
                                                mybir.AxisListType.X,
                                                mybir.AluOpType.max,
                                                negate=True)
                        P = pP.tile([TBc, ext], F16, tag="P")
                        nc.scalar.activation(P, S,
                                             mybir.ActivationFunctionType.Exp,
                                             bias=negmax[:, :], scale=1.0,
                                             accum_out=zsum[:, :])
                        nc.vector.reciprocal(rz, zsum)
                        pso = ppsm.tile([TBc, Hc], F32, tag="small64")
                        for st in range(ext // 128):
                            ppt = pppt.tile([128, 128], F16, tag="pt")
                            nc.tensor.transpose(
                                ppt, P[:, st * 128:(st + 1) * 128], idb)
                            ptt = pPT.tile([128, 128], F16, tag="ptt")
                            nc.any.tensor_copy(ptt, ppt)
                            nc.tensor.matmul(
                                pso, ptt,
                                vnat[:, (b * NST + st) * Hc:
                                     (b * NST + st + 1) * Hc],
                                start=(st == 0), stop=(st == ext // 128 - 1))
                        osb = po.tile([TBc, Hc], F32, tag="osb")
                        nc.vector.tensor_scalar_mul(osb, pso, rz[:, :])
                        nc.sync.dma_start(out.ap()[b, blk], osb)

    nc.compile()
    return nc


def _split(a):
    hi = np.asarray(a, dtype=np.float32).astype(ml_dtypes.bfloat16)
    lo = (np.asarray(a, dtype=np.float32) - hi.astype(np.float32)).astype(
        ml_dtypes.bfloat16)
    return hi, lo


def kernel(x, Wk, bk, Wq, bq, Wv, rel_pos_emb, mask, **_unused):
    global LAST_EXEC_NS
    x = np.asarray(x, dtype=np.float32)
    Wk = np.asarray(Wk, dtype=np.float32)
    bk = np.asarray(bk, dtype=np.float32)
    Wq = np.asarray(Wq, dtype=np.float32)
    bq = np.asarray(bq, dtype=np.float32)
    Wv = np.asarray(Wv, dtype=np.float32)
    rel = np.asarray(rel_pos_emb, dtype=np.float32)
    causal = bool(np.asarray(mask).item())
    cfg = _cfg(causal)
    exts = cfg["exts"]

    scale = np.float32(np.sqrt(H))
    # xT: [D, B, T]
    xT = np.ascontiguousarray(x.transpose(2, 0, 1))
    xh, xl = _split(xT)
    wkh, wkl = _split(Wk * scale)
    wqh, wql = _split(Wq)
    wvh = Wv.astype(ml_dtypes.bfloat16)
    bk8 = (bk * scale).reshape(H, 1).astype(np.float32)
    bqr = bq.reshape(H, 1).astype(np.float32)
    # relT: [T, H, T] (t, c, v) — single bf16 (no hi/lo split)
    relT = np.ascontiguousarray(rel.transpose(0, 2, 1))
    rth = relT.astype(ml_dtypes.bfloat16)
    identf = np.eye(64, dtype=np.float32)
    identb = np.eye(128, dtype=np.float16)

    in_maps = []
    blocks = []
    for c in range(NCORES):
        bA, bB = c, NBLK - 1 - c
        blocks.append((bA, bB))
        tiles = []
        for slot, blkid in ((0, bA), (1, bB)):
            rblk = rth[blkid * TB:(blkid + 1) * TB]       # [128, H, T]
            for g in range(TB // 8):
                rows = rblk[8 * g:8 * g + 8]              # [8, H, T]
                for ch in range(exts[slot] // 512):
                    t = rows[:, :, ch * 512:(ch + 1) * 512]
                    t = (t.reshape(4, 2, H, 512)
                         .transpose(1, 2, 0, 3).reshape(128, 2048))
                    tiles.append(t)
        relh_c = np.stack(tiles)
        xqh_c = np.stack([xh[:, :, bA * TB:(bA + 1) * TB],
                          xh[:, :, bB * TB:(bB + 1) * TB]], axis=1)
        xql_c = np.stack([xl[:, :, bA * TB:(bA + 1) * TB],
                          xl[:, :, bB * TB:(bB + 1) * TB]], axis=1)
        masks = []
        for slot, blkid in ((0, bA), (1, bB)):
            ext = exts[slot]
            t_idx = blkid * TB + np.arange(TB)[:, None]
            s_idx = np.arange(ext)[None, :]
            if causal:
                m = np.where(s_idx <= t_idx, 0.0, NEG)
            else:
                m = np.zeros((TB, ext))
            masks.append(np.ascontiguousarray(
                m.astype(ml_dtypes.float8_e4m3)))
        in_maps.append({
            "xh": np.ascontiguousarray(xh[:, c, :]),
            "xl": np.ascontiguousarray(xl[:, c, :]),
            "xqh": np.ascontiguousarray(xqh_c),
            "xql": np.ascontiguousarray(xql_c),
            "wkh": wkh, "wkl": wkl, "wqh": wqh, "wql": wql, "wv": wvh,
            "bk8": bk8, "bq": bqr,
            "relh": np.ascontiguousarray(relh_c),
            "maskA": masks[0], "maskB": masks[1],
            "identf": identf, "identb": identb,
        })

    nc = build_nc(cfg)
    if os.environ.get("KERNEL_TRACE") == "1":
        # the profile hook needs an initialized backend; trigger init first.
        _install_ntff_hook()
        import jax
        jax.devices()
        try:
            res = run_bass_kernel_spmd(
                nc, in_maps, core_ids=list(range(NCORES)), trace=True)
        except Exception:
            res = run_bass_kernel_spmd(
                nc, in_maps, core_ids=list(range(NCORES)))
    else:
        res = run_bass_kernel_spmd(nc, in_maps, core_ids=list(range(NCORES)))
    LAST_EXEC_NS = res.exec_time_ns
    global LAST_RES
    LAST_RES = res

    out = np.empty((B, T, H), dtype=np.float32)
    for c in range(NCORES):
        oc = res.results[c]["out"]          # [B, 2, TB, H]
        bA, bB = blocks[c]
        out[:, bA * TB:(bA + 1) * TB] = oc[:, 0]
        out[:, bB * TB:(bB + 1) * TB] = oc[:, 1]
    return out

